# revision 7
# baseline (speedup 1.0000x reference)
"""Trainium2 Bass kernel for nn_MultiHeadAttention_65773129171319.

Complex-valued multi-head attention:
  attn = softmax(|Qc Kc^H| / sqrt(2 dk)) ; out = (attn @ Vr) Wo, (attn @ Vp) Wo

Sharding: 8 cores = 2 (batch) x 4 (head-groups of 2 heads).  Each core
computes its batch's full sequence for its 2 heads; the out-projection
partial sums (over head groups) are reduced on the host.

Device algorithm (per core, all matmuls bf16, fp32 PSUM accumulation):
  - inputs arrive pre-transposed on host: X^T [D, S] per tensor, bf16
  - Q/K projections produce "stacked" transposed tiles per head:
      qc[h]  = [Qr_h^T ; Qp_h^T]        [128, S]
      kcr[h] = [Kr_h^T ; -Kp_h^T]       [128, S]
      kcp[h] = [Kp_h^T ;  Kr_h^T]       [128, S]
    so that the real/phase score matrices come out of single
    128-contraction matmuls, TRANSPOSED [sk, sq]:
      sT_r[sk,sq] = sum_c kcr[c,sk] qc[c,sq],  sT_p likewise with kcp.
  - u = sT_r^2 + sT_p^2 (ACT Square / custom DVE fused square-add),
    m = sqrt(u) (ACT), attn = exp(m/SCALE) (ACT, bf16 out).  Transposed
    scores let attn feed the AV matmul directly as the moving operand.
  - softmax denominators: ones-stationary matmul -> rowsums on partition 0,
    reciprocal (custom DVE approx), GPSIMD partition-broadcast, applied
    while copying the AV output out of PSUM.
  - AV output is stacked per head into xr2hT [128, S] (head h writes PSUM
    partitions h*64..), so the out-projection is one 128-contraction matmul.
"""

import os
import sys

import numpy as np

try:
    import concourse.bass as bass
except ImportError:  # pragma: no cover
    sys.path.insert(0, "/opt/trn_rl_repo")
    import concourse.bass as bass

import ml_dtypes
import concourse.mybir as mybir
import concourse.tile as tile
from concourse import bacc
from concourse.bass_utils import run_bass_kernel_spmd

B, S, D, H = 2, 2048, 512, 8
DK = D // H  # 64
SCALE = float((2 * DK) ** 0.5)
P = 128
N_CORES = 8
HG = 4            # head groups (2 heads each)
DT = D // P       # 4 d-tiles for projection contraction
SKT = S // P      # 16 sk tiles
NSTRIP = 4        # sq strips of 512
STRIP = S // NSTRIP  # 512

F32 = mybir.dt.float32
BF16 = mybir.dt.bfloat16
BFNP = ml_dtypes.bfloat16

AF = mybir.ActivationFunctionType


def register_custom_ops():
    """Register fused DVE ops (runtime extension of dve_ops.OPS)."""
    import concourse.dve_ops as dve_ops
    from concourse.dve_ops import DveOp
    from concourse.dve_spec import Spec, Src0, Src1, sq, lower, _has_src1
    from concourse.dve_uop import DveOpSpec

    existing = {op.name: op for op in dve_ops.OPS}

    def mk(name, spec):
        if name in existing:
            return existing[name]
        row = max(dve_ops._SUB_OPCODE_FOR_NAME.values()) + 1
        assert row < 0x20, "no free DVE opcode rows"
        dve_ops._SUB_OPCODE_FOR_NAME[name] = row
        shas = {}
        for ver in ("v3", "v4"):
            s = DveOpSpec(name=name, opcode=row, uops=lower(spec, ver=ver),
                          rd1_en=_has_src1(spec))
            shas[ver] = s.sha(ver)
        op = DveOp(name, spec, subdim=False, uops_sha=shas)
        dve_ops.OPS.append(op)
        return op

    sq1 = mk("SQ1_ANT", Spec(
        body=sq(Src0),
        reference=lambda in0, in1, s0, s1, imm2: in0.astype(np.float32) ** 2))
    sqadd = mk("SQADD_ANT", Spec(
        body=sq(Src0) + Src1,
        reference=lambda in0, in1, s0, s1, imm2:
            in0.astype(np.float32) ** 2 + in1.astype(np.float32)))
    return sq1, sqadd


SQ1, SQADD = register_custom_ops()


def build(n_iter: int = 1):
    """Build (and bacc-compile) the per-core SPMD program."""
    nc = bacc.Bacc("TRN2", target_bir_lowering=False, debug=False,
                   num_devices=N_CORES)

    dr = {}
    for name in ("xqr", "xqp", "xkr", "xkp", "xvr", "xvp"):
        dr[name] = nc.dram_tensor(name, [D, S], BF16, kind="ExternalInput")
    for name in ("wq", "wk", "wv"):
        dr[name] = nc.dram_tensor(name, [D, 2 * DK], BF16, kind="ExternalInput")
    dr["wo"] = nc.dram_tensor("wo", [2 * DK, D], BF16, kind="ExternalInput")
    dr["o_r"] = nc.dram_tensor("o_r", [S, D], F32, kind="ExternalOutput")
    dr["o_p"] = nc.dram_tensor("o_p", [S, D], F32, kind="ExternalOutput")

    with tile.TileContext(nc) as tc:
        _emit(tc, dr, n_iter)
    nc.compile()
    return nc


def _emit(tc, dr, n_iter):
    from contextlib import ExitStack

    ctx = ExitStack()
    with ctx:
        pools = dict(
            singles=ctx.enter_context(tc.tile_pool(name="singles", bufs=1)),
            xpool=ctx.enter_context(tc.tile_pool(name="xp", bufs=3)),
            upool=ctx.enter_context(tc.tile_pool(name="up", bufs=3)),
            apool=ctx.enter_context(tc.tile_pool(name="ap", bufs=3)),
            tpool=ctx.enter_context(tc.tile_pool(name="tp", bufs=3)),
            opool=ctx.enter_context(tc.tile_pool(name="op", bufs=3)),
            psA=ctx.enter_context(tc.tile_pool(name="psA", bufs=3, space="PSUM")),
            psAV=ctx.enter_context(tc.tile_pool(name="psAV", bufs=3, space="PSUM")),
            psO=ctx.enter_context(tc.tile_pool(name="psO", bufs=2, space="PSUM")),
        )
        if n_iter > 1:
            with tc.For_i(0, n_iter, 1):
                _body(tc, dr, **pools)
        else:
            _body(tc, dr, **pools)


def _body(tc, dr, singles, xpool, upool, apool, tpool, opool, psA, psAV, psO):
    nc = tc.nc

    # ---- weights to SBUF -------------------------------------------------
    wsb = {}
    for name in ("wq", "wk", "wv"):
        t = singles.tile([P, DT, 2 * DK], BF16, tag=f"w_{name}", name=f"w_{name}")
        nc.sync.dma_start(out=t[:], in_=dr[name].rearrange("(dt p) m -> p dt m", p=P))
        wsb[name] = t
    wkn = singles.tile([P, DT, 2 * DK], BF16, tag="w_wkn", name="w_wkn")
    nc.scalar.mul(out=wkn[:], in_=wsb["wk"][:], mul=-1.0)
    wo = singles.tile([P, D], BF16, tag="w_wo", name="w_wo")
    nc.sync.dma_start(out=wo[:], in_=dr["wo"][:])
    ones = singles.tile([P, 1], BF16, tag="ones", name="ones")
    nc.vector.memset(ones[:], 1.0)

    # ---- persistent SBUF tensors ----------------------------------------
    qc = [singles.tile([P, S], BF16, tag=f"qc{h}", name=f"qc{h}") for h in range(2)]
    kcr = [singles.tile([P, S], BF16, tag=f"kcr{h}", name=f"kcr{h}") for h in range(2)]
    kcp = [singles.tile([P, S], BF16, tag=f"kcp{h}", name=f"kcp{h}") for h in range(2)]
    vtr = [singles.tile([P, SKT, DK], BF16, tag=f"vtr{h}", name=f"vtr{h}") for h in range(2)]
    vtp = [singles.tile([P, SKT, DK], BF16, tag=f"vtp{h}", name=f"vtp{h}") for h in range(2)]
    xr2hT = singles.tile([P, S], BF16, tag="xr2hT", name="xr2hT")
    xp2hT = singles.tile([P, S], BF16, tag="xp2hT", name="xp2hT")

    # ---- K projection ----------------------------------------------------
    for s in range(NSTRIP):
        ssl = slice(s * STRIP, (s + 1) * STRIP)
        xtr = xpool.tile([P, DT, STRIP], BF16, tag="xs", name="xs")
        nc.sync.dma_start(
            out=xtr[:],
            in_=dr["xkr"].rearrange("(dt p) s -> p dt s", p=P)[:, :, ssl])
        xtp = xpool.tile([P, DT, STRIP], BF16, tag="xs", name="xs")
        nc.sync.dma_start(
            out=xtp[:],
            in_=dr["xkp"].rearrange("(dt p) s -> p dt s", p=P)[:, :, ssl])
        for h in range(2):
            hsl = slice(h * DK, (h + 1) * DK)
            ps_kcr = psA.tile([P, STRIP], F32, tag="psA", name="psA")
            ps_kcp = psA.tile([P, STRIP], F32, tag="psA", name="psA")
            for dt in range(DT):
                st = (dt == 0)
                sp = (dt == DT - 1)
                nc.tensor.matmul(ps_kcr[0:DK, :], wsb["wk"][:, dt, hsl],
                                 xtr[:, dt, :], start=st, stop=sp)
                nc.tensor.matmul(ps_kcr[DK:P, :], wkn[:, dt, hsl],
                                 xtp[:, dt, :], start=st, stop=sp)
                nc.tensor.matmul(ps_kcp[0:DK, :], wsb["wk"][:, dt, hsl],
                                 xtp[:, dt, :], start=st, stop=sp)
                nc.tensor.matmul(ps_kcp[DK:P, :], wsb["wk"][:, dt, hsl],
                                 xtr[:, dt, :], start=st, stop=sp)
            nc.scalar.copy(out=kcr[h][:, ssl], in_=ps_kcr[:])
            nc.scalar.copy(out=kcp[h][:, ssl], in_=ps_kcp[:])

    # ---- Q projection ----------------------------------------------------
    for s in range(NSTRIP):
        ssl = slice(s * STRIP, (s + 1) * STRIP)
        xtr = xpool.tile([P, DT, STRIP], BF16, tag="xs", name="xs")
        nc.sync.dma_start(
            out=xtr[:],
            in_=dr["xqr"].rearrange("(dt p) s -> p dt s", p=P)[:, :, ssl])
        xtp = xpool.tile([P, DT, STRIP], BF16, tag="xs", name="xs")
        nc.sync.dma_start(
            out=xtp[:],
            in_=dr["xqp"].rearrange("(dt p) s -> p dt s", p=P)[:, :, ssl])
        for h in range(2):
            hsl = slice(h * DK, (h + 1) * DK)
            ps_q = psA.tile([P, STRIP], F32, tag="psA", name="psA")
            for dt in range(DT):
                st = (dt == 0)
                sp = (dt == DT - 1)
                nc.tensor.matmul(ps_q[0:DK, :], wsb["wq"][:, dt, hsl],
                                 xtr[:, dt, :], start=st, stop=sp)
                nc.tensor.matmul(ps_q[DK:P, :], wsb["wq"][:, dt, hsl],
                                 xtp[:, dt, :], start=st, stop=sp)
            nc.scalar.copy(out=qc[h][:, ssl], in_=ps_q[:])

    # ---- V projection ----------------------------------------------------
    for kind, src, dst in (("r", "xvr", vtr), ("p", "xvp", vtp)):
        for s in range(NSTRIP):
            xt = xpool.tile([P, DT, STRIP], BF16, tag="xs", name="xs")
            nc.sync.dma_start(
                out=xt[:],
                in_=dr[src].rearrange("(dt p) s -> p dt s", p=P)[
                    :, :, s * STRIP:(s + 1) * STRIP])
            for tt in range(STRIP // P):
                t = s * (STRIP // P) + tt
                ps_v = psAV.tile([P, STRIP], F32, tag="av", name="av")
                for dt in range(DT):
                    nc.tensor.matmul(ps_v[:, 0:2 * DK],
                                     xt[:, dt, tt * P:(tt + 1) * P],
                                     wsb["wv"][:, dt, :],
                                     start=(dt == 0), stop=(dt == DT - 1))
                for h in range(2):
                    nc.vector.tensor_copy(dst[h][:, t, 0:DK],
                                          ps_v[:, h * DK:(h + 1) * DK])

    # ---- attention -------------------------------------------------------
    # loop: strip outer, head inner; both heads' u computed before the
    # sqrt/exp pair so ACT table switches batch (2 per strip).
    inv_scale = 1.0 / SCALE
    for s in range(NSTRIP):
        ssl = slice(s * STRIP, (s + 1) * STRIP)
        us = []
        for h in range(2):
            u = upool.tile([P, SKT, STRIP], BF16, tag="u", name="u")
            us.append(u)
            for t in range(SKT):
                tsl = slice(t * P, (t + 1) * P)
                ps_r = psA.tile([P, STRIP], F32, tag="psA", name="psA")
                nc.tensor.matmul(ps_r[:], kcr[h][:, tsl], qc[h][:, ssl],
                                 start=True, stop=True)
                ps_p = psA.tile([P, STRIP], F32, tag="psA", name="psA")
                nc.tensor.matmul(ps_p[:], kcp[h][:, tsl], qc[h][:, ssl],
                                 start=True, stop=True)
                if t % 2 == 0:
                    nc.scalar.square(u[:, t, :], ps_r[:])
                else:
                    nc.vector._custom_dve(SQ1, out=u[:, t, :], in0=ps_r[:])
                nc.vector._custom_dve(SQADD, out=u[:, t, :], in0=ps_p[:],
                                      in1=u[:, t, :])
        for h in range(2):
            nc.scalar.activation(us[h][:], us[h][:], AF.Sqrt)
        attns = []
        for h in range(2):
            attn = apool.tile([P, SKT, STRIP], BF16, tag="attn", name="attn")
            attns.append(attn)
            nc.scalar.activation(attn[:], us[h][:], AF.Exp, scale=inv_scale)
        for h in range(2):
            attn = attns[h]
            hps = slice(h * DK, (h + 1) * DK)
            # rowsums -> partition 0 (ones stationary, M=1)
            ps_rs = psAV.tile([P, STRIP], F32, tag="av", name="av")
            for t in range(SKT):
                nc.tensor.matmul(ps_rs[0:1, :], ones[:], attn[:, t, :],
                                 start=(t == 0), stop=(t == SKT - 1))
            rrec = tpool.tile([1, STRIP], F32, tag="rrec", name="rrec")
            nc.vector.reciprocal_approx_fast(rrec[:], ps_rs[0:1, :])
            rb = tpool.tile([P, STRIP], F32, tag="rb", name="rb")
            nc.gpsimd.partition_broadcast(rb[:], rrec[:])
            # AV: head h lands on PSUM partitions h*64..h*64+63
            ps_avr = psAV.tile([P, STRIP], F32, tag="av", name="av")
            for t in range(SKT):
                nc.tensor.matmul(ps_avr[hps, :], vtr[h][:, t, :],
                                 attn[:, t, :], start=(t == 0),
                                 stop=(t == SKT - 1))
            ps_avp = psAV.tile([P, STRIP], F32, tag="av", name="av")
            for t in range(SKT):
                nc.tensor.matmul(ps_avp[hps, :], vtp[h][:, t, :],
                                 attn[:, t, :], start=(t == 0),
                                 stop=(t == SKT - 1))
            nc.vector.tensor_mul(xr2hT[hps, ssl], ps_avr[hps, :], rb[hps, :])
            nc.vector.tensor_mul(xp2hT[hps, ssl], ps_avp[hps, :], rb[hps, :])

    # ---- out projection --------------------------------------------------
    for kind, xT, out in (("r", xr2hT, dr["o_r"]), ("p", xp2hT, dr["o_p"])):
        for q in range(S // P):
            qsl = slice(q * P, (q + 1) * P)
            ps_o = psO.tile([P, D], F32, tag="o", name="o")
            nc.tensor.matmul(ps_o[:], xT[:, qsl], wo[:], start=True, stop=True)
            osb = opool.tile([P, D], F32, tag="osb", name="osb")
            nc.scalar.copy(out=osb[:], in_=ps_o[:])
            nc.sync.dma_start(out=out[qsl, :], in_=osb[:])


# ---------------------------------------------------------------------------
_CACHE = {}


def _get_nc(n_iter=1):
    if n_iter not in _CACHE:
        _CACHE[n_iter] = build(n_iter)
    return _CACHE[n_iter]


def make_in_maps(q_real, k_real, v_real, q_phase, k_phase, v_phase,
                 w_q, w_k, w_v, w_o):
    """Host-side shard + layout prep: per-core input dicts."""
    xt = {}
    for b in range(B):
        xt[("xqr", b)] = np.ascontiguousarray(q_real[b].T).astype(BFNP)
        xt[("xqp", b)] = np.ascontiguousarray(q_phase[b].T).astype(BFNP)
        xt[("xkr", b)] = np.ascontiguousarray(k_real[b].T).astype(BFNP)
        xt[("xkp", b)] = np.ascontiguousarray(k_phase[b].T).astype(BFNP)
        xt[("xvr", b)] = np.ascontiguousarray(v_real[b].T).astype(BFNP)
        xt[("xvp", b)] = np.ascontiguousarray(v_phase[b].T).astype(BFNP)
    wq16, wk16, wv16, wo16 = (w.astype(BFNP) for w in (w_q, w_k, w_v, w_o))
    in_maps = []
    for core in range(N_CORES):
        b, hg = divmod(core, HG)
        csl = slice(hg * 2 * DK, (hg + 1) * 2 * DK)
        in_maps.append({
            "xqr": xt[("xqr", b)], "xqp": xt[("xqp", b)],
            "xkr": xt[("xkr", b)], "xkp": xt[("xkp", b)],
            "xvr": xt[("xvr", b)], "xvp": xt[("xvp", b)],
            "wq": np.ascontiguousarray(wq16[:, csl]),
            "wk": np.ascontiguousarray(wk16[:, csl]),
            "wv": np.ascontiguousarray(wv16[:, csl]),
            "wo": np.ascontiguousarray(wo16[csl, :]),
        })
    return in_maps


def gather_outputs(results):
    out_r = np.zeros((B, S, D), np.float32)
    out_p = np.zeros((B, S, D), np.float32)
    for core in range(N_CORES):
        b = core // HG
        out_r[b] += results[core]["o_r"]
        out_p[b] += results[core]["o_p"]
    return out_r, out_p


def _numpy_fallback(q_real, k_real, v_real, q_phase, k_phase, v_phase,
                    w_q, w_k, w_v, w_o, mask):
    def heads(x, w):
        y = x @ w
        return y.reshape(B, -1, H, DK).transpose(0, 2, 1, 3)
    qr, kr, vr = heads(q_real, w_q), heads(k_real, w_k), heads(v_real, w_v)
    qp, kp, vp = heads(q_phase, w_q), heads(k_phase, w_k), heads(v_phase, w_v)
    ar = np.einsum('bhqd,bhkd->bhqk', qr, kr) - np.einsum('bhqd,bhkd->bhqk', qp, kp)
    ap = np.einsum('bhqd,bhkd->bhqk', qr, kp) + np.einsum('bhqd,bhkd->bhqk', qp, kr)
    a = np.sqrt(ar * ar + ap * ap) / SCALE
    a = np.where(mask[:, None, :, :] == 0, np.float32(-1e9), a)
    a = a - a.max(axis=-1, keepdims=True)
    e = np.exp(a)
    a = e / e.sum(axis=-1, keepdims=True)
    xr = np.einsum('bhqk,bhkd->bhqd', a, vr).transpose(0, 2, 1, 3).reshape(B, -1, D)
    xp = np.einsum('bhqk,bhkd->bhqd', a, vp).transpose(0, 2, 1, 3).reshape(B, -1, D)
    return (xr @ w_o).astype(np.float32), (xp @ w_o).astype(np.float32)


def kernel(q_real, k_real, v_real, q_phase, k_phase, v_phase,
           w_q, w_k, w_v, w_o, mask):
    args = [np.asarray(a, np.float32) for a in
            (q_real, k_real, v_real, q_phase, k_phase, v_phase,
             w_q, w_k, w_v, w_o)]
    mask = np.asarray(mask)
    if not np.all(mask != 0):
        return _numpy_fallback(*args, mask)
    nc = _get_nc(1)
    in_maps = make_in_maps(*args)
    res = run_bass_kernel_spmd(nc, in_maps, core_ids=list(range(N_CORES)))
    return gather_outputs(res.results)


# revision 11
# speedup vs baseline: 32.4621x; 32.4621x over previous
"""Trainium2 Bass kernel for nn_MultiHeadAttention_65773129171319.

Complex-valued multi-head attention:
  attn = softmax(|Qc Kc^H| / sqrt(2 dk)) ; out = (attn @ Vr) Wo, (attn @ Vp) Wo

Sharding: 8 cores = 2 (batch) x 4 (head-groups of 2 heads).  Each core
computes its batch's full sequence for its 2 heads; the out-projection
partial sums (over head groups) are reduced on the host.

Device algorithm (per core, all matmuls bf16, fp32 PSUM accumulation):
  - inputs arrive pre-transposed on host: X^T [D, S] per tensor, bf16
  - Q/K projections produce "stacked" transposed tiles per head:
      qc[h]  = [Qr_h^T ; Qp_h^T]        [128, S]
      kcr[h] = [Kr_h^T ; -Kp_h^T]       [128, S]
      kcp[h] = [Kp_h^T ;  Kr_h^T]       [128, S]
    so that the real/phase score matrices come out of single
    128-contraction matmuls, TRANSPOSED [sk, sq]:
      sT_r[sk,sq] = sum_c kcr[c,sk] qc[c,sq],  sT_p likewise with kcp.
  - u = sT_r^2 + sT_p^2: ACT Square(ps_r) then custom fused DVE op
    SQADD (u = ps_p^2 + u), pipelining ACT against DVE; m = sqrt(u) (ACT),
    attn = exp(m/SCALE) (ACT, bf16 out).  sqrt/exp batch per strip across
    both heads to minimise ACT table-set switches.  Transposed scores let
    attn feed the AV matmul directly as the moving operand.
  - softmax denominators: ones-stationary matmul -> rowsums on partition 0,
    reciprocal (custom DVE approx), GPSIMD partition-broadcast, applied
    while copying the AV output out of PSUM.
  - AV output is stacked per head into xr2hT [128, S] (head h writes PSUM
    partitions h*64..), so the out-projection is one 128-contraction matmul.
"""

import os
import sys

import numpy as np

try:
    import concourse.bass as bass
except ImportError:  # pragma: no cover
    sys.path.insert(0, "/opt/trn_rl_repo")
    import concourse.bass as bass

import ml_dtypes
import concourse.mybir as mybir
import concourse.tile as tile
from concourse import bacc
from concourse.bass_utils import run_bass_kernel_spmd

B, S, D, H = 2, 2048, 512, 8
DK = D // H  # 64
SCALE = float((2 * DK) ** 0.5)
P = 128
N_CORES = 8
HG = 4            # head groups (2 heads each)
DT = D // P       # 4 d-tiles for projection contraction
SKT = S // P      # 16 sk tiles
NSTRIP = 4        # sq strips of 512
STRIP = S // NSTRIP  # 512

F32 = mybir.dt.float32
BF16 = mybir.dt.bfloat16
BFNP = ml_dtypes.bfloat16

AF = mybir.ActivationFunctionType


def register_custom_ops():
    """Register fused DVE ops (runtime extension of dve_ops.OPS)."""
    import concourse.dve_ops as dve_ops
    from concourse.dve_ops import DveOp
    from concourse.dve_spec import Spec, Src0, Src1, sq, lower, _has_src1
    from concourse.dve_uop import DveOpSpec

    existing = {op.name: op for op in dve_ops.OPS}

    def mk(name, spec):
        if name in existing:
            return existing[name]
        row = max(dve_ops._SUB_OPCODE_FOR_NAME.values()) + 1
        assert row < 0x20, "no free DVE opcode rows"
        dve_ops._SUB_OPCODE_FOR_NAME[name] = row
        shas = {}
        for ver in ("v3", "v4"):
            s = DveOpSpec(name=name, opcode=row, uops=lower(spec, ver=ver),
                          rd1_en=_has_src1(spec))
            shas[ver] = s.sha(ver)
        op = DveOp(name, spec, subdim=False, uops_sha=shas)
        dve_ops.OPS.append(op)
        return op

    sq1 = mk("SQ1_ANT", Spec(
        body=sq(Src0),
        reference=lambda in0, in1, s0, s1, imm2: in0.astype(np.float32) ** 2))
    sqadd = mk("SQADD_ANT", Spec(
        body=sq(Src0) + Src1,
        reference=lambda in0, in1, s0, s1, imm2:
            in0.astype(np.float32) ** 2 + in1.astype(np.float32)))
    return sq1, sqadd


SQ1, SQADD = register_custom_ops()


def build(n_iter: int = 1, variant: frozenset = frozenset()):
    """Build (and bacc-compile) the per-core SPMD program."""
    nc = bacc.Bacc("TRN2", target_bir_lowering=False, debug=False,
                   num_devices=N_CORES)

    dr = {}
    for name in ("xqr", "xqp", "xkr", "xkp", "xvr", "xvp"):
        dr[name] = nc.dram_tensor(name, [D, S], BF16, kind="ExternalInput")
    for name in ("wq", "wk", "wv"):
        dr[name] = nc.dram_tensor(name, [D, 2 * DK], BF16, kind="ExternalInput")
    dr["wo"] = nc.dram_tensor("wo", [2 * DK, D], BF16, kind="ExternalInput")
    dr["o_r"] = nc.dram_tensor("o_r", [S, D], F32, kind="ExternalOutput")
    dr["o_p"] = nc.dram_tensor("o_p", [S, D], F32, kind="ExternalOutput")

    with tile.TileContext(nc) as tc:
        _emit(tc, dr, n_iter, variant)
    nc.compile()
    return nc


def _emit(tc, dr, n_iter, variant=frozenset()):
    from contextlib import ExitStack

    ctx = ExitStack()
    with ctx:
        pools = dict(
            singles=ctx.enter_context(tc.tile_pool(name="singles", bufs=1)),
            xpool=ctx.enter_context(tc.tile_pool(name="xp", bufs=3)),
            upool=ctx.enter_context(tc.tile_pool(name="up", bufs=3)),
            apool=ctx.enter_context(tc.tile_pool(name="ap", bufs=3)),
            tpool=ctx.enter_context(tc.tile_pool(name="tp", bufs=3)),
            opool=ctx.enter_context(tc.tile_pool(name="op", bufs=3)),
            psA=ctx.enter_context(tc.tile_pool(name="psA", bufs=4, space="PSUM")),
            psAV=ctx.enter_context(tc.tile_pool(name="psAV", bufs=2, space="PSUM")),
            psO=ctx.enter_context(tc.tile_pool(name="psO", bufs=2, space="PSUM")),
        )
        if n_iter > 1:
            with tc.For_i(0, n_iter, 1):
                _body(tc, dr, variant, **pools)
        else:
            _body(tc, dr, variant, **pools)


def _body(tc, dr, variant, singles, xpool, upool, apool, tpool, opool, psA, psAV, psO):
    nc = tc.nc

    # ---- weights to SBUF -------------------------------------------------
    wsb = {}
    for name in ("wq", "wk", "wv"):
        t = singles.tile([P, DT, 2 * DK], BF16, tag=f"w_{name}", name=f"w_{name}")
        nc.sync.dma_start(out=t[:], in_=dr[name].rearrange("(dt p) m -> p dt m", p=P))
        wsb[name] = t
    wkn = singles.tile([P, DT, 2 * DK], BF16, tag="w_wkn", name="w_wkn")
    nc.scalar.mul(out=wkn[:], in_=wsb["wk"][:], mul=-1.0)
    wo = singles.tile([P, D], BF16, tag="w_wo", name="w_wo")
    nc.sync.dma_start(out=wo[:], in_=dr["wo"][:])
    ones = singles.tile([P, 1], BF16, tag="ones", name="ones")
    nc.vector.memset(ones[:], 1.0)

    # ---- persistent SBUF tensors ----------------------------------------
    qc = [singles.tile([P, S], BF16, tag=f"qc{h}", name=f"qc{h}") for h in range(2)]
    kcr = [singles.tile([P, S], BF16, tag=f"kcr{h}", name=f"kcr{h}") for h in range(2)]
    kcp = [singles.tile([P, S], BF16, tag=f"kcp{h}", name=f"kcp{h}") for h in range(2)]
    vtr = [singles.tile([P, SKT, DK], BF16, tag=f"vtr{h}", name=f"vtr{h}") for h in range(2)]
    vtp = [singles.tile([P, SKT, DK], BF16, tag=f"vtp{h}", name=f"vtp{h}") for h in range(2)]
    xr2hT = singles.tile([P, S], BF16, tag="xr2hT", name="xr2hT")
    xp2hT = singles.tile([P, S], BF16, tag="xp2hT", name="xp2hT")

    def _xdma(out, in_):
        if "nodma" not in variant:
            nc.sync.dma_start(out=out, in_=in_)

    # ---- K projection ----------------------------------------------------
    for s in range(NSTRIP):
        ssl = slice(s * STRIP, (s + 1) * STRIP)
        xtr = xpool.tile([P, DT, STRIP], BF16, tag="xs", name="xs")
        _xdma(xtr[:], dr["xkr"].rearrange("(dt p) s -> p dt s", p=P)[:, :, ssl])
        xtp = xpool.tile([P, DT, STRIP], BF16, tag="xs", name="xs")
        _xdma(xtp[:], dr["xkp"].rearrange("(dt p) s -> p dt s", p=P)[:, :, ssl])
        for h in range(2):
            if "noproj" in variant:
                break
            hsl = slice(h * DK, (h + 1) * DK)
            ps_kcr = psA.tile([P, STRIP], F32, tag="psA", name="psA")
            ps_kcp = psA.tile([P, STRIP], F32, tag="psA", name="psA")
            for dt in range(DT):
                st = (dt == 0)
                sp = (dt == DT - 1)
                nc.tensor.matmul(ps_kcr[0:DK, :], wsb["wk"][:, dt, hsl],
                                 xtr[:, dt, :], start=st, stop=sp)
                nc.tensor.matmul(ps_kcr[DK:P, :], wkn[:, dt, hsl],
                                 xtp[:, dt, :], start=st, stop=sp)
                nc.tensor.matmul(ps_kcp[0:DK, :], wsb["wk"][:, dt, hsl],
                                 xtp[:, dt, :], start=st, stop=sp)
                nc.tensor.matmul(ps_kcp[DK:P, :], wsb["wk"][:, dt, hsl],
                                 xtr[:, dt, :], start=st, stop=sp)
            nc.scalar.copy(out=kcr[h][:, ssl], in_=ps_kcr[:])
            nc.scalar.copy(out=kcp[h][:, ssl], in_=ps_kcp[:])

    # ---- Q projection ----------------------------------------------------
    for s in range(NSTRIP):
        ssl = slice(s * STRIP, (s + 1) * STRIP)
        xtr = xpool.tile([P, DT, STRIP], BF16, tag="xs", name="xs")
        _xdma(xtr[:], dr["xqr"].rearrange("(dt p) s -> p dt s", p=P)[:, :, ssl])
        xtp = xpool.tile([P, DT, STRIP], BF16, tag="xs", name="xs")
        _xdma(xtp[:], dr["xqp"].rearrange("(dt p) s -> p dt s", p=P)[:, :, ssl])
        for h in range(2):
            if "noproj" in variant:
                break
            hsl = slice(h * DK, (h + 1) * DK)
            ps_q = psA.tile([P, STRIP], F32, tag="psA", name="psA")
            for dt in range(DT):
                st = (dt == 0)
                sp = (dt == DT - 1)
                nc.tensor.matmul(ps_q[0:DK, :], wsb["wq"][:, dt, hsl],
                                 xtr[:, dt, :], start=st, stop=sp)
                nc.tensor.matmul(ps_q[DK:P, :], wsb["wq"][:, dt, hsl],
                                 xtp[:, dt, :], start=st, stop=sp)
            nc.scalar.copy(out=qc[h][:, ssl], in_=ps_q[:])

    # ---- V projection ----------------------------------------------------
    for kind, src, dst in (("r", "xvr", vtr), ("p", "xvp", vtp)):
        for s in range(NSTRIP):
            xt = xpool.tile([P, DT, STRIP], BF16, tag="xs", name="xs")
            _xdma(xt[:], dr[src].rearrange("(dt p) s -> p dt s", p=P)[
                :, :, s * STRIP:(s + 1) * STRIP])
            for tt in range(STRIP // P):
                if "noproj" in variant:
                    break
                t = s * (STRIP // P) + tt
                ps_v = psAV.tile([P, STRIP], F32, tag="av", name="av")
                for dt in range(DT):
                    nc.tensor.matmul(ps_v[:, 0:2 * DK],
                                     xt[:, dt, tt * P:(tt + 1) * P],
                                     wsb["wv"][:, dt, :],
                                     start=(dt == 0), stop=(dt == DT - 1))
                for h in range(2):
                    nc.vector.tensor_copy(dst[h][:, t, 0:DK],
                                          ps_v[:, h * DK:(h + 1) * DK])

    # ---- attention -------------------------------------------------------
    # loop: strip outer, head inner; both heads' u computed before the
    # sqrt/exp pair so ACT table switches batch (2 per strip).
    inv_scale = 1.0 / SCALE
    for s in range(NSTRIP):
        ssl = slice(s * STRIP, (s + 1) * STRIP)
        us = []
        for h in range(2):
            u = upool.tile([P, SKT, STRIP], BF16, tag="u", name="u")
            us.append(u)
            for t in range(SKT):
                tsl = slice(t * P, (t + 1) * P)
                ps_r = psA.tile([P, STRIP], F32, tag="psA", name="psA")
                if "noscores" not in variant:
                    nc.tensor.matmul(ps_r[:], kcr[h][:, tsl], qc[h][:, ssl],
                                     start=True, stop=True)
                ps_p = psA.tile([P, STRIP], F32, tag="psA", name="psA")
                if "noscores" not in variant:
                    nc.tensor.matmul(ps_p[:], kcp[h][:, tsl], qc[h][:, ssl],
                                     start=True, stop=True)
                if "nosq" in variant:
                    if t == 0:
                        nc.vector.memset(u[:], 0.25)
                    continue
                nc.scalar.square(u[:, t, :], ps_r[:])
                nc.vector._custom_dve(SQADD, out=u[:, t, :], in0=ps_p[:],
                                      in1=u[:, t, :])
        if "nosqrtexp" not in variant:
            for h in range(2):
                nc.scalar.activation(us[h][:], us[h][:], AF.Sqrt)
        attns = []
        for h in range(2):
            attn = apool.tile([P, SKT, STRIP], BF16, tag="attn", name="attn")
            attns.append(attn)
            if "nosqrtexp" in variant:
                nc.vector.tensor_copy(attn[:], us[h][:])
            else:
                nc.scalar.activation(attn[:], us[h][:], AF.Exp, scale=inv_scale)
        for h in range(2):
            attn = attns[h]
            hps = slice(h * DK, (h + 1) * DK)
            # rowsums -> partition 0 (ones stationary, M=1)
            ps_rs = psAV.tile([P, STRIP], F32, tag="av", name="av")
            if "norowsum" not in variant:
                for t in range(SKT):
                    nc.tensor.matmul(ps_rs[0:1, :], ones[:], attn[:, t, :],
                                     start=(t == 0), stop=(t == SKT - 1))
            rrec = tpool.tile([1, STRIP], F32, tag="rrec", name="rrec")
            if "norecip" in variant:
                nc.vector.memset(rrec[:], 1.0)
            else:
                nc.vector.reciprocal_approx_fast(rrec[:], ps_rs[0:1, :])
            rb = tpool.tile([P, STRIP], F32, tag="rb", name="rb")
            if "nobcast" in variant:
                nc.vector.memset(rb[:], 1.0)
            else:
                nc.gpsimd.partition_broadcast(rb[:], rrec[:])
            # AV: head h lands on PSUM partitions h*64..h*64+63
            ps_avr = psAV.tile([P, STRIP], F32, tag="av", name="av")
            if "noav" not in variant:
                for t in range(SKT):
                    nc.tensor.matmul(ps_avr[hps, :], vtr[h][:, t, :],
                                     attn[:, t, :], start=(t == 0),
                                     stop=(t == SKT - 1))
            ps_avp = psAV.tile([P, STRIP], F32, tag="av", name="av")
            if "noav" not in variant:
                for t in range(SKT):
                    nc.tensor.matmul(ps_avp[hps, :], vtp[h][:, t, :],
                                     attn[:, t, :], start=(t == 0),
                                     stop=(t == SKT - 1))
            nc.vector.tensor_mul(xr2hT[hps, ssl], ps_avr[hps, :], rb[hps, :])
            nc.vector.tensor_mul(xp2hT[hps, ssl], ps_avp[hps, :], rb[hps, :])

    # ---- out projection --------------------------------------------------
    for kind, xT, out in (("r", xr2hT, dr["o_r"]), ("p", xp2hT, dr["o_p"])):
        if "noout" in variant:
            break
        for q in range(S // P):
            qsl = slice(q * P, (q + 1) * P)
            ps_o = psO.tile([P, D], F32, tag="o", name="o")
            nc.tensor.matmul(ps_o[:], xT[:, qsl], wo[:], start=True, stop=True)
            osb = opool.tile([P, D], F32, tag="osb", name="osb")
            nc.vector.tensor_copy(osb[:], ps_o[:])
            nc.sync.dma_start(out=out[qsl, :], in_=osb[:])


# ---------------------------------------------------------------------------
_CACHE = {}


def _get_nc(n_iter=1, variant=frozenset()):
    key = (n_iter, variant)
    if key not in _CACHE:
        _CACHE[key] = build(n_iter, variant)
    return _CACHE[key]


def make_in_maps(q_real, k_real, v_real, q_phase, k_phase, v_phase,
                 w_q, w_k, w_v, w_o):
    """Host-side shard + layout prep: per-core input dicts."""
    xt = {}
    for b in range(B):
        xt[("xqr", b)] = np.ascontiguousarray(q_real[b].T).astype(BFNP)
        xt[("xqp", b)] = np.ascontiguousarray(q_phase[b].T).astype(BFNP)
        xt[("xkr", b)] = np.ascontiguousarray(k_real[b].T).astype(BFNP)
        xt[("xkp", b)] = np.ascontiguousarray(k_phase[b].T).astype(BFNP)
        xt[("xvr", b)] = np.ascontiguousarray(v_real[b].T).astype(BFNP)
        xt[("xvp", b)] = np.ascontiguousarray(v_phase[b].T).astype(BFNP)
    wq16, wk16, wv16, wo16 = (w.astype(BFNP) for w in (w_q, w_k, w_v, w_o))
    in_maps = []
    for core in range(N_CORES):
        b, hg = divmod(core, HG)
        csl = slice(hg * 2 * DK, (hg + 1) * 2 * DK)
        in_maps.append({
            "xqr": xt[("xqr", b)], "xqp": xt[("xqp", b)],
            "xkr": xt[("xkr", b)], "xkp": xt[("xkp", b)],
            "xvr": xt[("xvr", b)], "xvp": xt[("xvp", b)],
            "wq": np.ascontiguousarray(wq16[:, csl]),
            "wk": np.ascontiguousarray(wk16[:, csl]),
            "wv": np.ascontiguousarray(wv16[:, csl]),
            "wo": np.ascontiguousarray(wo16[csl, :]),
        })
    return in_maps


def gather_outputs(results):
    out_r = np.zeros((B, S, D), np.float32)
    out_p = np.zeros((B, S, D), np.float32)
    for core in range(N_CORES):
        b = core // HG
        out_r[b] += results[core]["o_r"]
        out_p[b] += results[core]["o_p"]
    return out_r, out_p


def _numpy_fallback(q_real, k_real, v_real, q_phase, k_phase, v_phase,
                    w_q, w_k, w_v, w_o, mask):
    def heads(x, w):
        y = x @ w
        return y.reshape(B, -1, H, DK).transpose(0, 2, 1, 3)
    qr, kr, vr = heads(q_real, w_q), heads(k_real, w_k), heads(v_real, w_v)
    qp, kp, vp = heads(q_phase, w_q), heads(k_phase, w_k), heads(v_phase, w_v)
    ar = np.einsum('bhqd,bhkd->bhqk', qr, kr) - np.einsum('bhqd,bhkd->bhqk', qp, kp)
    ap = np.einsum('bhqd,bhkd->bhqk', qr, kp) + np.einsum('bhqd,bhkd->bhqk', qp, kr)
    a = np.sqrt(ar * ar + ap * ap) / SCALE
    a = np.where(mask[:, None, :, :] == 0, np.float32(-1e9), a)
    a = a - a.max(axis=-1, keepdims=True)
    e = np.exp(a)
    a = e / e.sum(axis=-1, keepdims=True)
    xr = np.einsum('bhqk,bhkd->bhqd', a, vr).transpose(0, 2, 1, 3).reshape(B, -1, D)
    xp = np.einsum('bhqk,bhkd->bhqd', a, vp).transpose(0, 2, 1, 3).reshape(B, -1, D)
    return (xr @ w_o).astype(np.float32), (xp @ w_o).astype(np.float32)


def kernel(q_real, k_real, v_real, q_phase, k_phase, v_phase,
           w_q, w_k, w_v, w_o, mask):
    args = [np.asarray(a, np.float32) for a in
            (q_real, k_real, v_real, q_phase, k_phase, v_phase,
             w_q, w_k, w_v, w_o)]
    mask = np.asarray(mask)
    if not np.all(mask != 0):
        return _numpy_fallback(*args, mask)
    nc = _get_nc(1)
    in_maps = make_in_maps(*args)
    res = run_bass_kernel_spmd(nc, in_maps, core_ids=list(range(N_CORES)))
    return gather_outputs(res.results)
